# revision 5
# baseline (speedup 1.0000x reference)
"""Chamfer-distance (CDLoss) Trainium2 Bass kernel.

Problem: srcs, tgts [B=8, D=3, N=4096] fp32.
  P[b,i,j] = |s_i|^2 + |t_j|^2 - 2 s_i.t_j
  out = min(P, axis=1).mean() + min(P, axis=2).mean()   (scalar fp32)

Strategy (data-parallel over B across 8 NeuronCores, one batch per core):
  Two "directions" per core, each a 4096x4096 implicit distance matrix:
    dir1: for each source i, min over targets j of (|t_j|^2 - 2 s_i.t_j)
    dir2: for each target j, min over sources i of (|s_i|^2 - 2 t_j.s_i)
  The query-side norm (|s_i|^2 resp. |t_j|^2) is added on the host - it is
  constant per row so it commutes with the row-min.

  The matrix entries are produced by TensorE matmuls with bf16 hi/lo-split
  features (K=15 rows -> exact to ~1e-6 absolute), 512 columns per matmul
  into PSUM fp32.  Row-mins are computed by VectorE tensor_tensor_reduce
  (op0=min elementwise, op1=min reduce, chained via the scalar initial
  value): each TTR consumes 1024 fresh PSUM elements + 1024 SBUF elements
  (pre-copied from PSUM by ScalarE) per partition -> 2 elements/cycle/lane,
  the DVE I/O ceiling for fp32.

  Per-core outputs are the 2x[128, 32] row-min matrices; the host adds the
  query norms, averages, and combines across cores.
"""

import numpy as np
import ml_dtypes

_BF16 = ml_dtypes.bfloat16

# Problem geometry (hardcoded per contest contract).
_B = 8
_D = 3
_N = 4096
_P = 128              # partitions / queries per M-tile
_MT = _N // _P        # 32 M-tiles
_K = 15               # feature rows (bf16 hi/lo split, see _features)
_NCORES = 8

_prog_cache = {}

# test-harness knobs (the grading harness just calls kernel() and never
# touches these; default is the fast no-trace path)
TRACE = False
TRACE_CORES = [0]
LAST_RESULTS = None


def _build_program(n_pts=_N):
    import concourse.mybir as mybir
    import concourse.tile as tile
    from concourse import bacc

    P = _P
    MT = n_pts // P
    K = _K
    f32 = mybir.dt.float32
    bf16 = mybir.dt.bfloat16
    MIN = mybir.AluOpType.min

    nc = bacc.Bacc("TRN2", target_bir_lowering=False, debug=False,
                   num_devices=_NCORES)

    dram = {}
    for d in (1, 2):
        dram[f"w{d}"] = nc.dram_tensor(f"w{d}", [K, n_pts], bf16,
                                       kind="ExternalInput")
        dram[f"r{d}"] = nc.dram_tensor(f"r{d}", [K, n_pts], bf16,
                                       kind="ExternalInput")
        dram[f"out{d}"] = nc.dram_tensor(f"out{d}", [P, MT], f32,
                                         kind="ExternalOutput")

    with tile.TileContext(nc) as tc:
        with (
            tc.tile_pool(name="const", bufs=2) as cpool,
            tc.tile_pool(name="work", bufs=4) as wpool,
            tc.tile_pool(name="acc", bufs=2) as apool,
            tc.tile_pool(name="psum", bufs=2, space="PSUM") as ppool,
        ):
            for d in (1, 2):
                sbW = cpool.tile([K, n_pts], bf16, tag="sbW")
                nc.sync.dma_start(sbW[:], dram[f"w{d}"][:])
                sbR = cpool.tile([K, n_pts], bf16, tag="sbR")
                nc.sync.dma_start(sbR[:], dram[f"r{d}"][:])
                sbOut = cpool.tile([P, MT], f32, tag="sbOut")

                n_chunks = max(1, n_pts // 2048)
                assert n_chunks in (1, 2)
                for m in range(MT):
                    tmp = apool.tile([P, 1], f32, tag="tmp")
                    for h in range(n_chunks):
                        first, last = h == 0, h == n_chunks - 1
                        ps = ppool.tile([P, 2048], f32, tag="ps")
                        for q in range(4):
                            col = 2048 * h + 512 * q
                            nc.tensor.matmul(
                                ps[:, 512 * q:512 * (q + 1)],
                                sbW[:, m * P:(m + 1) * P],
                                sbR[:, col:col + 512],
                                start=True, stop=True,
                            )
                        # ScalarE stages the upper half into SBUF so the
                        # scan below can stream 2 elements/cycle (one via
                        # the PSUM read port, one via SBUF).
                        sbB = wpool.tile([P, 1024], f32, tag="sbB")
                        nc.scalar.copy(sbB[:], ps[:, 1024:2048])
                        # Running-min scan; broadcast-out means the last
                        # write (= min of everything + initial) lands in
                        # the [P,1] target.
                        tgt = sbOut[:, m:m + 1] if last else tmp[:]
                        nc.vector.tensor_tensor_scan(
                            out=tgt.broadcast_to((P, 1024)),
                            data0=ps[:, 0:1024],
                            data1=sbB[:],
                            initial=(3.0e38 if first else tmp[:]),
                            op0=MIN,
                            op1=MIN,
                        )
                nc.sync.dma_start(dram[f"out{d}"][:], sbOut[:])

    nc.compile()
    return nc


def _get_program(n_pts=_N):
    if n_pts not in _prog_cache:
        _prog_cache[n_pts] = _build_program(n_pts)
    return _prog_cache[n_pts]


def _split_bf16(x32):
    """x32 fp32 -> (hi, lo) bf16 with hi+lo ~= x to ~2^-18 rel."""
    hi = x32.astype(_BF16)
    lo = (x32 - hi.astype(np.float32)).astype(_BF16)
    return hi, lo


def _features(q, c):
    """Feature tensors for one direction.

    q: query points  [3, N] fp32 (their norm is added host-side)
    c: candidate points [3, N] fp32
    Returns (W [15, N] bf16, R [15, N] bf16, q_norm [N] f64) with
      (W.T @ R)[i, j] ~= |c~_j|^2 - 2 q~_i . c~_j     (~ = bf16-split value)
    """
    q_hi, q_lo = _split_bf16(q)
    c_hi, c_lo = _split_bf16(c)
    q_t = q_hi.astype(np.float32) + q_lo.astype(np.float32)
    c_t = c_hi.astype(np.float32) + c_lo.astype(np.float32)

    U = (c_t.astype(np.float64) ** 2).sum(axis=0)
    u_hi = U.astype(_BF16)
    res = U - u_hi.astype(np.float64)
    u_lo = res.astype(_BF16)
    res2 = res - u_lo.astype(np.float64)
    u_lo2 = res2.astype(_BF16)

    m2q_hi = (-2.0 * q_hi.astype(np.float32)).astype(_BF16)
    m2q_lo = (-2.0 * q_lo.astype(np.float32)).astype(_BF16)
    ones = np.ones_like(U, dtype=_BF16)

    W = np.concatenate([
        m2q_hi, m2q_hi, m2q_lo, m2q_lo,
        np.stack([ones, ones, ones]),
    ], axis=0).astype(_BF16)
    R = np.concatenate([
        c_hi, c_lo, c_hi, c_lo,
        np.stack([u_hi, u_lo, u_lo2]),
    ], axis=0).astype(_BF16)

    q_norm = (q_t.astype(np.float64) ** 2).sum(axis=0)
    return W, R, q_norm


def kernel(srcs, tgts):
    import concourse.bass_utils as bass_utils

    srcs = np.asarray(srcs, dtype=np.float32)
    tgts = np.asarray(tgts, dtype=np.float32)
    B = srcs.shape[0]
    assert srcs.shape == (B, _D, _N) and tgts.shape == (B, _D, _N)

    nc = _get_program()

    in_maps = []
    s_norms, t_norms = [], []
    for b in range(B):
        s = srcs[b]
        t = tgts[b]
        W1, R1, s_norm = _features(s, t)   # dir1: queries = sources
        W2, R2, t_norm = _features(t, s)   # dir2: queries = targets
        in_maps.append({"w1": W1, "r1": R1, "w2": W2, "r2": R2})
        s_norms.append(s_norm)
        t_norms.append(t_norm)

    res = bass_utils.run_bass_kernel_spmd(
        nc, in_maps, core_ids=list(range(_NCORES)),
        trace=TRACE, trace_cores=TRACE_CORES if TRACE else None,
    )
    global LAST_RESULTS
    LAST_RESULTS = res

    total = 0.0
    for b in range(B):
        out1 = res.results[b]["out1"]   # [128, 32]; query i = m*128 + p
        out2 = res.results[b]["out2"]
        min1 = out1.T.reshape(-1).astype(np.float64) + s_norms[b]
        min2 = out2.T.reshape(-1).astype(np.float64) + t_norms[b]
        # reference: min(P, axis=1).mean() -> per-target mins (dir2);
        #            min(P, axis=2).mean() -> per-source mins (dir1)
        total += min2.mean() + min1.mean()

    return np.float32(total / B)


# revision 7
# speedup vs baseline: 1.5209x; 1.5209x over previous
"""Chamfer-distance (CDLoss) Trainium2 Bass kernel.

Problem: srcs, tgts [B=8, D=3, N=4096] fp32.
  P[b,i,j] = |s_i|^2 + |t_j|^2 - 2 s_i.t_j
  out = min(P, axis=1).mean() + min(P, axis=2).mean()   (scalar fp32)

Strategy (data-parallel over B across 8 NeuronCores, one batch per core):
  Two "directions" per core, each a 4096x4096 implicit distance matrix:
    dir1: for each source i, min over targets j of d^2(s_i, t_j)
    dir2: for each target j, min over sources i of d^2(t_j, s_i)

  Matrix tiles are produced by TensorE matmuls with bf16 hi/lo-split
  features (K=18 rows; query and candidate norms are folded in, so PSUM
  holds d^2 >= 0 exactly to ~1e-6).  The 4 matmuls of each PSUM chunk go
  to 4 different PE row groups (tile_position) so LDWEIGHTS/MATMUL
  overlap and matmuls run concurrently.

  Row-min reduction is split between two engines:
    - "assisted" M-tiles: ScalarE casts PSUM fp32 -> fp16 SBUF (ACTIVATE
      Copy), then VectorE runs a tensor_tensor min tree in fp16 (2x packed
      mode, 2 outputs/cycle) + one small 1x reduce.
    - "pure" M-tiles: VectorE reduces PSUM fp32 directly at 1x.
  The ratio keeps both engines saturated.

  Per-core outputs are 2x[128, 32] row-min matrices; the host averages
  and combines across cores (query norms are already included).
"""

import numpy as np
import ml_dtypes

_BF16 = ml_dtypes.bfloat16

# Problem geometry (hardcoded per contest contract).
_B = 8
_D = 3
_N = 4096
_P = 128              # partitions / queries per M-tile
_K = 18               # feature rows (see _features)
_NCORES = 8
_CHUNK = 1024         # PSUM chunk columns (2 banks)

_prog_cache = {}

# test-harness knobs (the grading harness just calls kernel() and never
# touches these; default is the fast no-trace path)
TRACE = False
TRACE_CORES = [0]
LAST_RESULTS = None

# Which M-tiles take the DVE-only path (the rest are ACT-assisted).
_PURE_EVERY = 5       # m % _PURE_EVERY == 2 -> pure-DVE tile


def _build_program(n_pts=_N):
    import concourse.mybir as mybir
    import concourse.tile as tile
    from concourse import bacc

    P = _P
    MT = n_pts // P
    K = _K
    NCH = n_pts // _CHUNK          # psum chunks per M-tile
    f32 = mybir.dt.float32
    f16 = mybir.dt.float16
    bf16 = mybir.dt.bfloat16
    MIN = mybir.AluOpType.min

    nc = bacc.Bacc("TRN2", target_bir_lowering=False, debug=False,
                   num_devices=_NCORES)

    dram = {}
    for d in (1, 2):
        dram[f"w{d}"] = nc.dram_tensor(f"w{d}", [128, n_pts], bf16,
                                       kind="ExternalInput")
        dram[f"r{d}"] = nc.dram_tensor(f"r{d}", [128, n_pts], bf16,
                                       kind="ExternalInput")
        dram[f"out{d}"] = nc.dram_tensor(f"out{d}", [P, MT], f32,
                                         kind="ExternalOutput")

    with tile.TileContext(nc) as tc:
        with (
            tc.tile_pool(name="const", bufs=2) as cpool,
            tc.tile_pool(name="work", bufs=3) as wpool,
            tc.tile_pool(name="tree", bufs=3) as tpool,
            tc.tile_pool(name="acc", bufs=2) as apool,
            tc.tile_pool(name="psum", bufs=4, space="PSUM") as ppool,
        ):
            for d in (1, 2):
                sbW = cpool.tile([128, n_pts], bf16, tag="sbW")
                nc.sync.dma_start(sbW[:], dram[f"w{d}"][:])
                sbR = cpool.tile([128, n_pts], bf16, tag="sbR")
                nc.sync.dma_start(sbR[:], dram[f"r{d}"][:])
                sbOut = cpool.tile([P, MT], f32, tag="sbOut")

                for m in range(MT):
                    pure = (m % _PURE_EVERY == 2)
                    chunks = []
                    for h in range(NCH):
                        ps = ppool.tile([P, _CHUNK], f32, tag="ps")
                        for q in range(2):
                            # row group rotates per 512-col matmul so
                            # LDWEIGHTS overlaps in-flight MATMULs and
                            # matmuls run concurrently on the PE array.
                            g = (2 * h + q) % 4
                            col = _CHUNK * h + 512 * q
                            nc.tensor.matmul(
                                ps[:, 512 * q:512 * (q + 1)],
                                sbW[32 * g:32 * g + K, m * P:(m + 1) * P],
                                sbR[32 * g:32 * g + K, col:col + 512],
                                start=True, stop=True,
                                tile_position=(32 * g, 0),
                            )
                        chunks.append(ps)

                    if pure:
                        tmp = apool.tile([P, NCH], f32, tag="tmp")
                        for h, ps in enumerate(chunks):
                            nc.vector.tensor_reduce(
                                tmp[:, h:h + 1], ps[:],
                                axis=mybir.AxisListType.X, op=MIN)
                        nc.vector.tensor_reduce(
                            sbOut[:, m:m + 1], tmp[:],
                            axis=mybir.AxisListType.X, op=MIN)
                    else:
                        sb = []
                        for h, ps in enumerate(chunks):
                            cast = wpool.tile([P, _CHUNK], f16,
                                              tag=f"cast{h}")
                            nc.scalar.copy(cast[:], ps[:])
                            sb.append(cast)
                        # fp16 min tree on DVE (2x packed mode): fold
                        # tile pairs, then halve widths down to 512.
                        lvl = 0
                        while len(sb) > 1:
                            nxt = []
                            for i in range(0, len(sb), 2):
                                o = tpool.tile([P, _CHUNK], f16,
                                               tag=f"t{lvl}_{i}")
                                nc.vector.tensor_tensor(
                                    o[:], sb[i][:], sb[i + 1][:], op=MIN)
                                nxt.append(o)
                            sb = nxt
                            lvl += 1
                        last = tpool.tile([P, _CHUNK // 2], f16, tag="last")
                        nc.vector.tensor_tensor(
                            last[:], sb[0][:, :_CHUNK // 2],
                            sb[0][:, _CHUNK // 2:], op=MIN)
                        nc.vector.tensor_reduce(
                            sbOut[:, m:m + 1], last[:],
                            axis=mybir.AxisListType.X, op=MIN)
                nc.sync.dma_start(dram[f"out{d}"][:], sbOut[:])

    nc.compile()
    return nc


def _get_program(n_pts=_N):
    if n_pts not in _prog_cache:
        _prog_cache[n_pts] = _build_program(n_pts)
    return _prog_cache[n_pts]


def _split_bf16(x32):
    """x32 fp32 -> (hi, lo) bf16 with hi+lo ~= x to ~2^-18 rel."""
    hi = x32.astype(_BF16)
    lo = (x32 - hi.astype(np.float32)).astype(_BF16)
    return hi, lo


def _split3(x64):
    """fp64 vector -> 3 bf16 terms summing to x to ~2^-27 rel."""
    t0 = x64.astype(_BF16)
    r = x64 - t0.astype(np.float64)
    t1 = r.astype(_BF16)
    r2 = r - t1.astype(np.float64)
    t2 = r2.astype(_BF16)
    return t0, t1, t2


def _features(q, c, n_pts):
    """Feature tensors for one direction, replicated into 4 row groups.

    q: query points  [3, N] fp32; c: candidate points [3, N] fp32.
    Returns (W [128, N] bf16, R [128, N] bf16) such that for each row
    group g (partitions 32g..32g+17):
      (W[32g:32g+18].T @ R[32g:32g+18])[i, j] ~= |q~_i - c~_j|^2
    with ~ the bf16-split (hi+lo) values, exact to ~2e-6.
    """
    q_hi, q_lo = _split_bf16(q)
    c_hi, c_lo = _split_bf16(c)
    q_t = q_hi.astype(np.float32) + q_lo.astype(np.float32)
    c_t = c_hi.astype(np.float32) + c_lo.astype(np.float32)

    U = (c_t.astype(np.float64) ** 2).sum(axis=0)   # candidate norms
    u0, u1, u2 = _split3(U)
    V = (q_t.astype(np.float64) ** 2).sum(axis=0)   # query norms
    v0, v1, v2 = _split3(V)

    m2q_hi = (-2.0 * q_hi.astype(np.float32)).astype(_BF16)
    m2q_lo = (-2.0 * q_lo.astype(np.float32)).astype(_BF16)
    ones = np.ones(n_pts, dtype=_BF16)

    Wg = np.concatenate([
        m2q_hi, m2q_hi, m2q_lo, m2q_lo,
        np.stack([ones, ones, ones]),
        np.stack([v0, v1, v2]),
    ], axis=0).astype(_BF16)              # [18, N]
    Rg = np.concatenate([
        c_hi, c_lo, c_hi, c_lo,
        np.stack([u0, u1, u2]),
        np.stack([ones, ones, ones]),
    ], axis=0).astype(_BF16)              # [18, N]

    W = np.zeros((128, n_pts), dtype=_BF16)
    R = np.zeros((128, n_pts), dtype=_BF16)
    for g in range(4):
        W[32 * g:32 * g + _K] = Wg
        R[32 * g:32 * g + _K] = Rg
    return W, R


def kernel(srcs, tgts):
    import concourse.bass_utils as bass_utils

    srcs = np.asarray(srcs, dtype=np.float32)
    tgts = np.asarray(tgts, dtype=np.float32)
    B = srcs.shape[0]
    assert srcs.shape == (B, _D, _N) and tgts.shape == (B, _D, _N)

    nc = _get_program()

    in_maps = []
    for b in range(B):
        s = srcs[b]
        t = tgts[b]
        W1, R1 = _features(s, t, _N)   # dir1: queries = sources
        W2, R2 = _features(t, s, _N)   # dir2: queries = targets
        in_maps.append({"w1": W1, "r1": R1, "w2": W2, "r2": R2})

    res = bass_utils.run_bass_kernel_spmd(
        nc, in_maps, core_ids=list(range(_NCORES)),
        trace=TRACE, trace_cores=TRACE_CORES if TRACE else None,
    )
    global LAST_RESULTS
    LAST_RESULTS = res

    total = 0.0
    for b in range(B):
        out1 = res.results[b]["out1"]   # [128, 32]; query i = m*128 + p
        out2 = res.results[b]["out2"]
        # reference: min(P, axis=1).mean() -> per-target mins (dir2);
        #            min(P, axis=2).mean() -> per-source mins (dir1)
        total += (out2.astype(np.float64).mean()
                  + out1.astype(np.float64).mean())

    return np.float32(total / B)
